# revision 5
# baseline (speedup 1.0000x reference)
"""CoAtten2 Trainium2 kernel: 8-way tensor-parallel over one TRN2 chip.

Reference computation (C=1024, H=W=64, HW=4096):
    q   = (Wq @ Xm + bq)  viewed [1024, 2048] then transposed
    kf  = (Wk1 @ Xf + bk1) viewed [1024, 2048]
    kl  = (Wk2 @ Xl + bk2) viewed [1024, 2048]
    att = softmax(kf @ q) + softmax(kl @ q)          # [1024, 1024]
    out = gamma * (att @ (Wv @ Xm + bv)) + (Xf + Xl)/2

Decomposition (per core d of 8; group t = d//4, rank r = d%4):
  - Channel indices are permuted (I' = 512t + o <-> i = 2o + t) so the
    torch-style reshape becomes contiguous; the permutation is folded into the
    host-side Wv/bv prep and the output DMA access pattern; gamma into Wv/bv.
  - logits_PERM splits into parity quadrants Q(t, t') whose kf operand needs
    spatial columns [2048t, 2048t+2048) and whose q operand needs spatial
    columns [2048t', ...). Core d owns spatial slice S_d = [512d, 512(d+1));
    it computes the partial contraction over S_d of Q(t, t'=0 and 1) for both
    attention branches from local projections.
  - All matmul operands are fp16 (1 cycle/row on the PE, vs ~2-4 for f32r);
    PSUM accumulation stays fp32.  CPU simulation of this exact quantization
    scheme gives rel_err 6.8e-3 (gate 2e-2).
  - Two 4-way fp16 ReduceScatters (one per branch) deal each core its 128-row
    logits block; the f-branch RS overlaps the l-branch compute, the l-branch
    RS overlaps the V projection + residual prep.
  - softmax is a free-dim reduction; summed attention is AllGathered in fp16;
    the output phase reads att^T back via DMA-transpose and accumulates
    att^T_chunk @ V_chunk per output tile, plus residual.
"""

import sys

sys.path.insert(0, "/opt/trn_rl_repo")

import numpy as np

import concourse.bacc as bacc
import concourse.mybir as mybir
from concourse import tile
from concourse.bass_utils import run_bass_kernel_spmd

F32 = mybir.dt.float32
F16 = mybir.dt.float16

C = 1024
HW = 4096
S = 512          # spatial columns per core
CH = 512         # C // 2 (projection output channels)
NCORES = 8

_CACHE: dict = {}


def _build():
    nc = bacc.Bacc("TRN2", target_bir_lowering=False, debug=False, num_devices=NCORES)

    # per-core external inputs (all fp16 data path; fp32 bias rows for adds)
    xm = nc.declare_dram_parameter("xm", [C, S], F16, isOutput=False)
    xf = nc.declare_dram_parameter("xf", [C, S], F16, isOutput=False)
    xl = nc.declare_dram_parameter("xl", [C, S], F16, isOutput=False)
    xq0 = nc.declare_dram_parameter("xq0", [C, S], F16, isOutput=False)  # Xm block d%4
    xq1 = nc.declare_dram_parameter("xq1", [C, S], F16, isOutput=False)  # Xm block 4+d%4
    wq = nc.declare_dram_parameter("wq", [C, CH], F16, isOutput=False)   # Wq.T
    wk1 = nc.declare_dram_parameter("wk1", [C, CH], F16, isOutput=False)
    wk2 = nc.declare_dram_parameter("wk2", [C, CH], F16, isOutput=False)
    wv = nc.declare_dram_parameter("wv", [C, C], F16, isOutput=False)    # (g*Wv)[permJ].T
    bqr = nc.declare_dram_parameter("bqr", [128, CH], F32, isOutput=False)
    bk1r = nc.declare_dram_parameter("bk1r", [128, CH], F32, isOutput=False)
    bk2r = nc.declare_dram_parameter("bk2r", [128, CH], F32, isOutput=False)
    bvp = nc.declare_dram_parameter("bvp", [128, 8], F32, isOutput=False)
    out_ext = nc.declare_dram_parameter("out", [C, S], F32, isOutput=True)

    # internal DRAM
    rs_in_f = nc.dram_tensor("rs_in_f", [CH, C], F16)   # quadrant partials
    rs_in_l = nc.dram_tensor("rs_in_l", [CH, C], F16)
    rs_out_f = nc.dram_tensor("rs_out_f", [128, C], F16)
    rs_out_l = nc.dram_tensor("rs_out_l", [128, C], F16)
    att_in = nc.dram_tensor("att_in", [128, C], F16)
    att_out = nc.dram_tensor("att_out", [C, C], F16, addr_space="Shared")

    groups8 = [list(range(NCORES))]
    groups4 = [[0, 1, 2, 3], [4, 5, 6, 7]]

    with tile.TileContext(nc) as tc:
        with (
            tc.tile_pool(name="pw", bufs=1) as pw,
            tc.tile_pool(name="psg", bufs=3) as psg,
            tc.tile_pool(name="psc", bufs=2) as psc,
            tc.tile_pool(name="pps", bufs=4, space="PSUM") as pps,
            tc.tile_pool(name="plog", bufs=2, space="PSUM") as plog,
        ):
            # ---- merged input loads: one DMA per tensor ---------------------
            # dram [1024, W] -> sbuf [128, 8*W]; chunk c lives at cols [W*c,)
            def load_x(dram, width, tag):
                t = pw.tile([128, 8 * width], F16, tag=tag)
                nc.sync.dma_start(
                    t[:].rearrange("p (c w) -> p c w", c=8),
                    dram[:].rearrange("(c p) w -> p c w", p=128),
                )
                return t

            def bias_tile(dram, tag):
                t = pw.tile([128, CH], F32, tag=tag)
                nc.sync.dma_start(t[:], dram[:, :])
                return t

            xf_t = load_x(xf, S, "xf")
            wk1_t = load_x(wk1, CH, "wk1")
            bk1_t = bias_tile(bk1r, "bk1")
            wq_t = load_x(wq, CH, "wq")
            bq_t = bias_tile(bqr, "bq")
            xq0_t = load_x(xq0, S, "xq0")
            xq1_t = load_x(xq1, S, "xq1")
            xl_t = load_x(xl, S, "xl")
            wk2_t = load_x(wk2, CH, "wk2")
            bk2_t = bias_tile(bk2r, "bk2")
            xm_t = load_x(xm, S, "xm")
            wv_t = load_x(wv, C, "wv")
            bv_t = pw.tile([128, 8], F32, tag="bv")
            nc.sync.dma_start(bv_t[:], bvp[:, :])

            # ---- local transposed projections -------------------------------
            # proj(X, WT, b)[s, o] = sum_c X[c, s] WT[c, o] + b[o]  -> [512, 512]
            # stays in SBUF as 4 [128, 512] fp16 tiles (s on partitions).
            def proj(x_t, w_t, b_t, otag):
                outs = []
                for ssub in range(4):
                    ps = pps.tile([128, CH], F32, tag="mm")
                    for c in range(8):
                        nc.tensor.matmul(
                            ps[:],
                            x_t[:, S * c + 128 * ssub:S * c + 128 * (ssub + 1)],
                            w_t[:, CH * c:CH * (c + 1)],
                            start=(c == 0),
                            stop=(c == 7),
                        )
                    o = pw.tile([128, CH], F16, tag=f"{otag}{ssub}")
                    nc.vector.tensor_add(o[:], ps[:], b_t[:])
                    outs.append(o)
                return outs

            def partials(ck, cq, rin):
                # o-tile m: partial[128 o, 512 t'-block] over local s
                for m in range(4):
                    psl = plog.tile([128, C], F32, tag="lg")
                    for tp in range(2):
                        for k in range(4):
                            nc.tensor.matmul(
                                psl[:, CH * tp:CH * (tp + 1)],
                                ck[k][:, 128 * m:128 * (m + 1)],
                                cq[tp][k][:],
                                start=(k == 0),
                                stop=(k == 3),
                            )
                    stg = psg.tile([128, C], F16, tag="stg")
                    nc.vector.tensor_copy(stg[:], psl[:])
                    nc.sync.dma_start(rin[128 * m:128 * (m + 1), :], stg[:])

            # f-branch chain first: its RS runs while the l-branch computes
            ckf = proj(xf_t, wk1_t, bk1_t, "ckf")
            cq0 = proj(xq0_t, wq_t, bq_t, "cq0")
            cq1 = proj(xq1_t, wq_t, bq_t, "cq1")
            cq = [cq0, cq1]

            partials(ckf, cq, rs_in_f)
            nc.gpsimd.collective_compute(
                "ReduceScatter",
                mybir.AluOpType.add,
                ins=[rs_in_f[:]],
                outs=[rs_out_f[:]],
                replica_groups=groups4,
            )

            ckl = proj(xl_t, wk2_t, bk2_t, "ckl")
            partials(ckl, cq, rs_in_l)
            nc.gpsimd.collective_compute(
                "ReduceScatter",
                mybir.AluOpType.add,
                ins=[rs_in_l[:]],
                outs=[rs_out_l[:]],
                replica_groups=groups4,
            )

            # ---- V projection (local, overlaps the RS/AG window) ------------
            # V[J', hw_d] fp16, bias per J' partition
            v_sb = []
            for j in range(8):
                ps = pps.tile([128, S], F32, tag="mm")
                for c in range(8):
                    nc.tensor.matmul(
                        ps[:],
                        wv_t[:, C * c + 128 * j:C * c + 128 * (j + 1)],
                        xm_t[:, S * c:S * (c + 1)],
                        start=(c == 0),
                        stop=(c == 7),
                    )
                v = pw.tile([128, S], F16, tag=f"v{j}")
                nc.vector.tensor_scalar_add(v[:], ps[:], bv_t[:, j:j + 1])
                v_sb.append(v)

            # ---- residual: R[e] = 0.5 * (xf + xl) on permuted rows ----------
            # permuted row (e, p) = dram row 2*(128*(e%4)+p) + e//4; one
            # strided DMA per input: [128, 4096] with e-blocks at cols 512e.
            def load_resid(dram, tag):
                t = pw.tile([128, 8 * S], F16, tag=tag)
                nc.sync.dma_start(
                    t[:].rearrange("p (two m w) -> p two m w", two=2, m=4),
                    dram[:].rearrange("(m p two) w -> p two m w", p=128, two=2),
                )
                return t

            rf_t = load_resid(xf, "rf")
            rl_t = load_resid(xl, "rl")
            r_sb = []
            for e in range(8):
                r = pw.tile([128, S], F32, tag=f"r{e}")
                nc.vector.tensor_add(
                    r[:], rf_t[:, S * e:S * (e + 1)], rl_t[:, S * e:S * (e + 1)]
                )
                nc.scalar.mul(r[:], r[:], 0.5)
                r_sb.append(r)

            # ---- softmax on the dealt 128-row block -------------------------
            att_parts = []
            for ci, rout in ((0, rs_out_f), (1, rs_out_l)):
                lg = pw.tile([128, C], F16, tag=f"lg{ci}")
                nc.sync.dma_start(lg[:], rout[:, :])
                mxn = psc.tile([128, 1], F32, tag="mx")
                nc.vector.reduce_max(
                    mxn[:], lg[:], axis=mybir.AxisListType.X, negate=True
                )
                sm = psc.tile([128, 1], F32, tag="sm")
                at = pw.tile([128, C], F16, tag=f"at{ci}")
                nc.scalar.activation(
                    at[:],
                    lg[:],
                    mybir.ActivationFunctionType.Exp,
                    bias=mxn[:, 0:1],
                    accum_out=sm[:, 0:1],
                )
                rcp = psc.tile([128, 1], F32, tag="rc")
                nc.vector.reciprocal(rcp[:], sm[:])
                nc.vector.tensor_scalar_mul(at[:], at[:], rcp[:, 0:1])
                att_parts.append(at)
            att_sum = pw.tile([128, C], F16, tag="atsum")
            nc.vector.tensor_add(att_sum[:], att_parts[0][:], att_parts[1][:])
            nc.sync.dma_start(att_in[:, :], att_sum[:])
            nc.gpsimd.collective_compute(
                "AllGather",
                mybir.AluOpType.bypass,
                ins=[att_in[:]],
                outs=[att_out[:]],
                replica_groups=groups8,
            )

            # ---- out[:, hw_d] = att @ V_d + R -------------------------------
            att_t = []
            for k in range(8):
                t = pw.tile([128, C], F16, tag=f"attt{k}")
                nc.sync.dma_start(
                    t[:], att_out[:, 128 * k:128 * (k + 1)], transpose=True
                )
                att_t.append(t)
            out_v = out_ext[:].rearrange("(o t) w -> t o w", t=2)
            for e in range(8):
                ps = pps.tile([128, S], F32, tag="mm")
                for k in range(8):
                    nc.tensor.matmul(
                        ps[:],
                        att_t[k][:, 128 * e:128 * (e + 1)],
                        v_sb[k][:],
                        start=(k == 0),
                        stop=(k == 7),
                    )
                ost = psg.tile([128, S], F32, tag="ost")
                nc.vector.tensor_add(ost[:], ps[:], r_sb[e][:])
                nc.sync.dma_start(
                    out_v[e // 4, 128 * (e % 4):128 * (e % 4 + 1), :], ost[:]
                )

    nc.compile()
    return nc


def _prep_inputs(x_f, x_m, x_l, Wq, bq, Wk1, bk1, Wk2, bk2, Wv, bv, gamma):
    Xf = np.ascontiguousarray(x_f.reshape(C, HW), dtype=np.float16)
    Xm = np.ascontiguousarray(x_m.reshape(C, HW), dtype=np.float16)
    Xl = np.ascontiguousarray(x_l.reshape(C, HW), dtype=np.float16)
    g = np.float64(np.asarray(gamma).reshape(-1)[0])

    permJ = 2 * (np.arange(C) % 512) + np.arange(C) // 512  # J' -> global j
    wv_full = np.ascontiguousarray(
        (g * Wv.astype(np.float64))[permJ, :].T, dtype=np.float16
    )
    bv_perm = (g * bv.astype(np.float64))[permJ].astype(np.float32)

    wq_full = np.ascontiguousarray(Wq.T, dtype=np.float16)
    wk1_full = np.ascontiguousarray(Wk1.T, dtype=np.float16)
    wk2_full = np.ascontiguousarray(Wk2.T, dtype=np.float16)
    bqr = np.ascontiguousarray(np.broadcast_to(bq, (128, CH)), dtype=np.float32)
    bk1r = np.ascontiguousarray(np.broadcast_to(bk1, (128, CH)), dtype=np.float32)
    bk2r = np.ascontiguousarray(np.broadcast_to(bk2, (128, CH)), dtype=np.float32)
    bvp = np.ascontiguousarray(bv_perm.reshape(8, 128).T)

    in_maps = []
    for d in range(NCORES):
        sl = slice(S * d, S * (d + 1))
        s0 = slice(S * (d % 4), S * (d % 4 + 1))
        s1 = slice(S * (4 + d % 4), S * (4 + d % 4 + 1))
        in_maps.append({
            "xm": np.ascontiguousarray(Xm[:, sl]),
            "xf": np.ascontiguousarray(Xf[:, sl]),
            "xl": np.ascontiguousarray(Xl[:, sl]),
            "xq0": np.ascontiguousarray(Xm[:, s0]),
            "xq1": np.ascontiguousarray(Xm[:, s1]),
            "wq": wq_full,
            "wk1": wk1_full,
            "wk2": wk2_full,
            "wv": wv_full,
            "bqr": bqr,
            "bk1r": bk1r,
            "bk2r": bk2r,
            "bvp": bvp,
        })
    return in_maps


def _run(inputs: dict, trace: bool = False, **kw):
    if "nc" not in _CACHE:
        _CACHE["nc"] = _build()
    nc = _CACHE["nc"]
    in_maps = _prep_inputs(**inputs)
    res = run_bass_kernel_spmd(nc, in_maps, list(range(NCORES)), trace=trace, **kw)
    out = np.empty((C, HW), np.float32)
    for d in range(NCORES):
        out[:, S * d:S * (d + 1)] = res.results[d]["out"]
    return out.reshape(1, C, 64, 64), res


def kernel(**inputs) -> np.ndarray:
    inputs = {k: np.asarray(v) for k, v in inputs.items()}
    out, _ = _run(inputs)
    return out


# revision 9
# speedup vs baseline: 1.0027x; 1.0027x over previous
"""CoAtten2 Trainium2 kernel: 8-way tensor-parallel over one TRN2 chip.

Reference computation (C=1024, H=W=64, HW=4096):
    q   = (Wq @ Xm + bq)  viewed [1024, 2048] then transposed
    kf  = (Wk1 @ Xf + bk1) viewed [1024, 2048]
    kl  = (Wk2 @ Xl + bk2) viewed [1024, 2048]
    att = softmax(kf @ q) + softmax(kl @ q)          # [1024, 1024]
    out = gamma * (att @ (Wv @ Xm + bv)) + (Xf + Xl)/2

Decomposition (per core d of 8; group t = d//4, rank r = d%4):
  - Channel indices are permuted (I' = 512t + o <-> i = 2o + t) so the
    torch-style reshape becomes contiguous; the permutation is folded into the
    host-side Wv/bv prep and the output DMA access pattern; gamma into Wv/bv.
  - logits_PERM splits into parity quadrants Q(t, t') whose kf operand needs
    spatial columns [2048t, 2048t+2048) and whose q operand needs spatial
    columns [2048t', ...). Core d owns spatial slice S_d = [512d, 512(d+1));
    it computes the partial contraction over S_d of Q(t, t'=0 and 1) for both
    attention branches from local projections.
  - All matmul operands are fp16 (1 cycle/row on the PE, vs ~2-4 for f32r);
    PSUM accumulation stays fp32.  CPU simulation of this exact quantization
    scheme gives rel_err 6.8e-3 (gate 2e-2).
  - Two 4-way fp16 ReduceScatters (one per branch) deal each core its 128-row
    logits block; the f-branch RS overlaps the l-branch compute, the l-branch
    RS overlaps the V projection + residual prep.
  - softmax is a free-dim reduction; summed attention is AllGathered in fp16;
    the output phase reads att^T back via DMA-transpose and accumulates
    att^T_chunk @ V_chunk per output tile, plus residual.
"""

import sys

sys.path.insert(0, "/opt/trn_rl_repo")

import numpy as np

import concourse.bacc as bacc
import concourse.mybir as mybir
from concourse import tile
from concourse.bass_utils import run_bass_kernel_spmd

F32 = mybir.dt.float32
F16 = mybir.dt.float16

C = 1024
HW = 4096
S = 512          # spatial columns per core
CH = 512         # C // 2 (projection output channels)
NCORES = 8

_CACHE: dict = {}


def _build():
    nc = bacc.Bacc("TRN2", target_bir_lowering=False, debug=False, num_devices=NCORES)

    # per-core external inputs (all fp16 data path; fp32 bias rows for adds)
    xm = nc.declare_dram_parameter("xm", [C, S], F16, isOutput=False)
    xf = nc.declare_dram_parameter("xf", [C, S], F16, isOutput=False)
    xl = nc.declare_dram_parameter("xl", [C, S], F16, isOutput=False)
    xq0 = nc.declare_dram_parameter("xq0", [C, S], F16, isOutput=False)  # Xm block d%4
    xq1 = nc.declare_dram_parameter("xq1", [C, S], F16, isOutput=False)  # Xm block 4+d%4
    wq = nc.declare_dram_parameter("wq", [C, CH], F16, isOutput=False)   # Wq.T
    wk1 = nc.declare_dram_parameter("wk1", [C, CH], F16, isOutput=False)
    wk2 = nc.declare_dram_parameter("wk2", [C, CH], F16, isOutput=False)
    wv = nc.declare_dram_parameter("wv", [C, C], F16, isOutput=False)    # (g*Wv)[permJ].T
    bqr = nc.declare_dram_parameter("bqr", [128, CH], F32, isOutput=False)
    bk1r = nc.declare_dram_parameter("bk1r", [128, CH], F32, isOutput=False)
    bk2r = nc.declare_dram_parameter("bk2r", [128, CH], F32, isOutput=False)
    bvp = nc.declare_dram_parameter("bvp", [128, 8], F32, isOutput=False)
    out_ext = nc.declare_dram_parameter("out", [C, S], F32, isOutput=True)

    # internal DRAM
    rs_in = nc.dram_tensor("rs_in", [2 * CH, C], F16)   # interleaved f/l partials
    rs_out = nc.dram_tensor("rs_out", [256, C], F16)
    att_in = nc.dram_tensor("att_in", [128, C], F16)
    att_out = nc.dram_tensor("att_out", [C, C], F16, addr_space="Shared")
    dmy_in = nc.dram_tensor("dmy_in", [128, 16], F16)
    dmy_out = nc.dram_tensor("dmy_out", [1024, 16], F16, addr_space="Shared")

    groups8 = [list(range(NCORES))]
    groups4 = [[0, 1, 2, 3], [4, 5, 6, 7]]

    with tile.TileContext(nc) as tc:
        with (
            tc.tile_pool(name="pw", bufs=1) as pw,
            tc.tile_pool(name="psg", bufs=3) as psg,
            tc.tile_pool(name="psc", bufs=2) as psc,
            tc.tile_pool(name="pps", bufs=4, space="PSUM") as pps,
            tc.tile_pool(name="plog", bufs=2, space="PSUM") as plog,
        ):
            # ---- absorb the collective rendezvous barrier early -------------
            # tiny AllGather with no data deps: triggers immediately, binds the
            # runtime's first-collective barrier + cold-op cost to a throwaway
            nc.gpsimd.collective_compute(
                "AllGather",
                mybir.AluOpType.bypass,
                ins=[dmy_in[:]],
                outs=[dmy_out[:]],
                replica_groups=groups8,
            )

            # ---- merged input loads: one DMA per tensor, spread over both
            # HWDGE rings (sync + scalar) ------------------------------------
            # dram [1024, W] -> sbuf [128, 8*W]; chunk c lives at cols [W*c,)
            def load_x(dram, width, tag, eng):
                t = pw.tile([128, 8 * width], F16, tag=tag)
                eng.dma_start(
                    t[:].rearrange("p (c w) -> p c w", c=8),
                    dram[:].rearrange("(c p) w -> p c w", p=128),
                )
                return t

            def bias_tile(dram, tag, eng):
                t = pw.tile([128, CH], F32, tag=tag)
                eng.dma_start(t[:], dram[:, :])
                return t

            xf_t = load_x(xf, S, "xf", nc.sync)
            wk1_t = load_x(wk1, CH, "wk1", nc.scalar)
            bk1_t = bias_tile(bk1r, "bk1", nc.scalar)
            wq_t = load_x(wq, CH, "wq", nc.scalar)
            bq_t = bias_tile(bqr, "bq", nc.scalar)
            xq0_t = load_x(xq0, S, "xq0", nc.sync)
            xq1_t = load_x(xq1, S, "xq1", nc.sync)
            xl_t = load_x(xl, S, "xl", nc.sync)
            wk2_t = load_x(wk2, CH, "wk2", nc.scalar)
            bk2_t = bias_tile(bk2r, "bk2", nc.scalar)
            xm_t = load_x(xm, S, "xm", nc.sync)
            wv_t = load_x(wv, C, "wv", nc.scalar)
            bv_t = pw.tile([128, 8], F32, tag="bv")
            nc.scalar.dma_start(bv_t[:], bvp[:, :])

            # ---- local transposed projections -------------------------------
            # proj(X, WT, b)[s, o] = sum_c X[c, s] WT[c, o] + b[o]  -> [512, 512]
            # stays in SBUF as 4 [128, 512] fp16 tiles (s on partitions).
            def proj(x_t, w_t, b_t, otag):
                outs = []
                for ssub in range(4):
                    ps = pps.tile([128, CH], F32, tag="mm")
                    for c in range(8):
                        nc.tensor.matmul(
                            ps[:],
                            x_t[:, S * c + 128 * ssub:S * c + 128 * (ssub + 1)],
                            w_t[:, CH * c:CH * (c + 1)],
                            start=(c == 0),
                            stop=(c == 7),
                        )
                    o = pw.tile([128, CH], F16, tag=f"{otag}{ssub}")
                    nc.vector.tensor_add(o[:], ps[:], b_t[:])
                    outs.append(o)
                return outs

            def partials(ck, cq, branch):
                # o-tile m: partial[128 o, 512 t'-block] over local s; f/l
                # branches interleave per 256-row chunk so the single RS deals
                # each rank its own (f, l) 128-row pair
                for m in range(4):
                    psl = plog.tile([128, C], F32, tag="lg")
                    for tp in range(2):
                        for k in range(4):
                            nc.tensor.matmul(
                                psl[:, CH * tp:CH * (tp + 1)],
                                ck[k][:, 128 * m:128 * (m + 1)],
                                cq[tp][k][:],
                                start=(k == 0),
                                stop=(k == 3),
                            )
                    stg = psg.tile([128, C], F16, tag="stg")
                    nc.vector.tensor_copy(stg[:], psl[:])
                    row = 256 * m + 128 * branch
                    nc.sync.dma_start(rs_in[row:row + 128, :], stg[:])

            # all projections first, then both branches' partials, so the
            # merged RS input is complete as early as possible
            ckf = proj(xf_t, wk1_t, bk1_t, "ckf")
            cq0 = proj(xq0_t, wq_t, bq_t, "cq0")
            cq1 = proj(xq1_t, wq_t, bq_t, "cq1")
            cq = [cq0, cq1]
            ckl = proj(xl_t, wk2_t, bk2_t, "ckl")

            partials(ckf, cq, 0)
            partials(ckl, cq, 1)
            nc.gpsimd.collective_compute(
                "ReduceScatter",
                mybir.AluOpType.add,
                ins=[rs_in[:]],
                outs=[rs_out[:]],
                replica_groups=groups4,
            )

            # ---- V projection (local, overlaps the RS/AG window) ------------
            # V[J', hw_d] fp16, bias per J' partition
            v_sb = []
            for j in range(8):
                ps = pps.tile([128, S], F32, tag="mm")
                for c in range(8):
                    nc.tensor.matmul(
                        ps[:],
                        wv_t[:, C * c + 128 * j:C * c + 128 * (j + 1)],
                        xm_t[:, S * c:S * (c + 1)],
                        start=(c == 0),
                        stop=(c == 7),
                    )
                v = pw.tile([128, S], F16, tag=f"v{j}")
                nc.vector.tensor_scalar_add(v[:], ps[:], bv_t[:, j:j + 1])
                v_sb.append(v)

            # ---- residual: R[e] = 0.5 * (xf + xl) on permuted rows ----------
            # permuted row (e, p) = dram row 2*(128*(e%4)+p) + e//4; one
            # strided DMA per input: [128, 4096] with e-blocks at cols 512e.
            def load_resid(dram, tag):
                t = pw.tile([128, 8 * S], F16, tag=tag)
                nc.sync.dma_start(
                    t[:].rearrange("p (two m w) -> p two m w", two=2, m=4),
                    dram[:].rearrange("(m p two) w -> p two m w", p=128, two=2),
                )
                return t

            rf_t = load_resid(xf, "rf")
            rl_t = load_resid(xl, "rl")
            r_sb = []
            for e in range(8):
                r = pw.tile([128, S], F32, tag=f"r{e}")
                nc.vector.tensor_add(
                    r[:], rf_t[:, S * e:S * (e + 1)], rl_t[:, S * e:S * (e + 1)]
                )
                nc.scalar.mul(r[:], r[:], 0.5)
                r_sb.append(r)

            # ---- softmax on the dealt (f, l) 128-row blocks -----------------
            att_parts = []
            for ci in (0, 1):
                lg = pw.tile([128, C], F16, tag=f"lg{ci}")
                nc.sync.dma_start(lg[:], rs_out[128 * ci:128 * (ci + 1), :])
                mxn = psc.tile([128, 1], F32, tag="mx")
                nc.vector.reduce_max(
                    mxn[:], lg[:], axis=mybir.AxisListType.X, negate=True
                )
                sm = psc.tile([128, 1], F32, tag="sm")
                at = pw.tile([128, C], F16, tag=f"at{ci}")
                nc.scalar.activation(
                    at[:],
                    lg[:],
                    mybir.ActivationFunctionType.Exp,
                    bias=mxn[:, 0:1],
                    accum_out=sm[:, 0:1],
                )
                rcp = psc.tile([128, 1], F32, tag="rc")
                nc.vector.reciprocal(rcp[:], sm[:])
                nc.vector.tensor_scalar_mul(at[:], at[:], rcp[:, 0:1])
                att_parts.append(at)
            att_sum = pw.tile([128, C], F16, tag="atsum")
            nc.vector.tensor_add(att_sum[:], att_parts[0][:], att_parts[1][:])
            nc.sync.dma_start(att_in[:, :], att_sum[:])
            nc.gpsimd.collective_compute(
                "AllGather",
                mybir.AluOpType.bypass,
                ins=[att_in[:]],
                outs=[att_out[:]],
                replica_groups=groups8,
            )

            # ---- out[:, hw_d] = att @ V_d + R -------------------------------
            att_t = []
            for k in range(8):
                t = pw.tile([128, C], F16, tag=f"attt{k}")
                nc.sync.dma_start(
                    t[:], att_out[:, 128 * k:128 * (k + 1)], transpose=True
                )
                att_t.append(t)
            out_v = out_ext[:].rearrange("(o t) w -> t o w", t=2)
            for e in range(8):
                ps = pps.tile([128, S], F32, tag="mm")
                for k in range(8):
                    nc.tensor.matmul(
                        ps[:],
                        att_t[k][:, 128 * e:128 * (e + 1)],
                        v_sb[k][:],
                        start=(k == 0),
                        stop=(k == 7),
                    )
                ost = psg.tile([128, S], F32, tag="ost")
                nc.vector.tensor_add(ost[:], ps[:], r_sb[e][:])
                nc.sync.dma_start(
                    out_v[e // 4, 128 * (e % 4):128 * (e % 4 + 1), :], ost[:]
                )

    nc.compile()
    return nc


def _prep_inputs(x_f, x_m, x_l, Wq, bq, Wk1, bk1, Wk2, bk2, Wv, bv, gamma):
    Xf = np.ascontiguousarray(x_f.reshape(C, HW), dtype=np.float16)
    Xm = np.ascontiguousarray(x_m.reshape(C, HW), dtype=np.float16)
    Xl = np.ascontiguousarray(x_l.reshape(C, HW), dtype=np.float16)
    g = np.float64(np.asarray(gamma).reshape(-1)[0])

    permJ = 2 * (np.arange(C) % 512) + np.arange(C) // 512  # J' -> global j
    wv_full = np.ascontiguousarray(
        (g * Wv.astype(np.float64))[permJ, :].T, dtype=np.float16
    )
    bv_perm = (g * bv.astype(np.float64))[permJ].astype(np.float32)

    wq_full = np.ascontiguousarray(Wq.T, dtype=np.float16)
    wk1_full = np.ascontiguousarray(Wk1.T, dtype=np.float16)
    wk2_full = np.ascontiguousarray(Wk2.T, dtype=np.float16)
    bqr = np.ascontiguousarray(np.broadcast_to(bq, (128, CH)), dtype=np.float32)
    bk1r = np.ascontiguousarray(np.broadcast_to(bk1, (128, CH)), dtype=np.float32)
    bk2r = np.ascontiguousarray(np.broadcast_to(bk2, (128, CH)), dtype=np.float32)
    bvp = np.ascontiguousarray(bv_perm.reshape(8, 128).T)

    in_maps = []
    for d in range(NCORES):
        sl = slice(S * d, S * (d + 1))
        s0 = slice(S * (d % 4), S * (d % 4 + 1))
        s1 = slice(S * (4 + d % 4), S * (4 + d % 4 + 1))
        in_maps.append({
            "xm": np.ascontiguousarray(Xm[:, sl]),
            "xf": np.ascontiguousarray(Xf[:, sl]),
            "xl": np.ascontiguousarray(Xl[:, sl]),
            "xq0": np.ascontiguousarray(Xm[:, s0]),
            "xq1": np.ascontiguousarray(Xm[:, s1]),
            "wq": wq_full,
            "wk1": wk1_full,
            "wk2": wk2_full,
            "wv": wv_full,
            "bqr": bqr,
            "bk1r": bk1r,
            "bk2r": bk2r,
            "bvp": bvp,
        })
    return in_maps


def _run(inputs: dict, trace: bool = False, **kw):
    if "nc" not in _CACHE:
        _CACHE["nc"] = _build()
    nc = _CACHE["nc"]
    in_maps = _prep_inputs(**inputs)
    res = run_bass_kernel_spmd(nc, in_maps, list(range(NCORES)), trace=trace, **kw)
    out = np.empty((C, HW), np.float32)
    for d in range(NCORES):
        out[:, S * d:S * (d + 1)] = res.results[d]["out"]
    return out.reshape(1, C, 64, 64), res


def kernel(**inputs) -> np.ndarray:
    inputs = {k: np.asarray(v) for k, v in inputs.items()}
    out, _ = _run(inputs)
    return out
